# revision 18
# baseline (speedup 1.0000x reference)
"""PrefSimMat kernel, fp8 DoubleRow variant with UNIFORM pair-K=128 matmuls.

Both matmuls per tile are DoubleRow [128, 2, *] so the PE tile mode never
changes (the earlier fp8 attempts interleaved a K=2/3 matmul, which
reconfigured the PE row groups every tile and pinned the clock cold).
Features contract in ONE DoubleRow matmul (128 partitions x 2 pairs = 256
rows); the nj extension rides a second DoubleRow matmul with 3 live rows
(nj - 256 = 16*hi + mid + lo/16 in fp8e4) zero-padded to the same shape.
"""

import numpy as np
import ml_dtypes

F8 = ml_dtypes.float8_e4m3

N = 8192
D = 256
P = 128
NCORES = 8
M_PER_CORE = N // NCORES
MC = M_PER_CORE // P
NT = 512
GW = 2048
GROUPS = [(0, 2048), (2048, 2048), (4096, 2048), (6144, 2048)]
NG = len(GROUPS)
EPS = 2.0 ** -3
CNJ = 256.0

OUT_DT = np.float16

_CACHE = {}


def _build_nc():
    import concourse.bass as bass
    import concourse.mybir as mybir

    f32 = mybir.dt.float32
    f16 = mybir.dt.float16
    f8 = mybir.dt.float8e4
    AF = mybir.ActivationFunctionType
    ALU = mybir.AluOpType
    PM = mybir.MatmulPerfMode.DoubleRow

    nc = bass.Bass()
    l_d = nc.dram_tensor("lt", [P, 2, M_PER_CORE], f8, kind="ExternalInput")
    r_d = nc.dram_tensor("rt", [P, 2, N], f8, kind="ExternalInput")
    extw_d = nc.dram_tensor("extw", [P, 2, P], f8, kind="ExternalInput")
    extr_d = nc.dram_tensor("extr", [2, 2, N], f8, kind="ExternalInput")
    sc_d = nc.dram_tensor("sc", [P, 4 * MC], f32, kind="ExternalInput")
    out_d = nc.dram_tensor("out", [M_PER_CORE, N], f16, kind="ExternalOutput")

    NGI = MC * NG

    from contextlib import ExitStack

    with ExitStack() as ctx:
        r_s = ctx.enter_context(nc.sbuf_tensor("r_s", [P, 2, N], f8))
        l_s = ctx.enter_context(nc.sbuf_tensor("l_s", [P, 2, M_PER_CORE], f8))
        extw_s = ctx.enter_context(nc.sbuf_tensor("extw_s", [P, 2, P], f8))
        extr_s = ctx.enter_context(nc.sbuf_tensor("extr_s", [P, 2, N], f8))
        sc_s = ctx.enter_context(nc.sbuf_tensor("sc_s", [P, 4 * MC], f32))
        tbuf = ctx.enter_context(nc.sbuf_tensor("tbuf", [P, 4 * GW], f16))
        stage = ctx.enter_context(nc.sbuf_tensor("stage", [P, 4 * GW], f16))
        ps = ctx.enter_context(nc.psum_tensor("ps", [P, 2 * GW], f32))
        rhs_g_sems = [
            [ctx.enter_context(nc.semaphore(f"in_r{c}_{g}")) for c in range(2)]
            for g in range(NG)
        ]
        in_l = ctx.enter_context(nc.semaphore("in_l"))
        in_ext = ctx.enter_context(nc.semaphore("in_ext"))
        in_sc = ctx.enter_context(nc.semaphore("in_sc"))
        sem_mm = ctx.enter_context(nc.semaphore("sem_mm"))
        sem_act = ctx.enter_context(nc.semaphore("sem_act"))
        sem_ts = ctx.enter_context(nc.semaphore("sem_ts"))
        out_sems = [ctx.enter_context(nc.semaphore(f"dma_o{s}")) for s in range(4)]
        sem_z = ctx.enter_context(nc.semaphore("sem_z"))
        sem_dsq = ctx.enter_context(nc.semaphore("sem_dsq"))
        block = ctx.enter_context(nc.Block())

        @block.sync
        def _(sync):
            sync.dma_start(l_s[:, :, :], l_d[:, :, :]).then_inc(in_l, 16)
            sync.dma_start(extw_s[:, :, :], extw_d[:, :, :]).then_inc(in_ext, 16)
            for g, (c0, w) in enumerate(GROUPS):
                sync.dma_start(
                    r_s[:, :, c0 : c0 + w], r_d[:, :, c0 : c0 + w]
                ).then_inc(rhs_g_sems[g][0], 16)
                # only the 2 live ext rows come from DRAM; the whole tensor
                # is zeroed first by the DVE memset (quadrant-aligned)
                if g == 0:
                    sync.wait_ge(sem_z, 1)
                elif g == NG // 2:
                    sync.wait_ge(sem_z, 2)
                sync.dma_start(
                    extr_s[0:2, :, c0 : c0 + w], extr_d[:, :, c0 : c0 + w]
                ).then_inc(rhs_g_sems[g][1], 16)
                if g == 0:
                    sync.dma_start(sc_s[:, :], sc_d[:, :]).then_inc(in_sc, 16)
            for u in range(NGI):
                g, m = divmod(u, MC)
                c0, w = GROUPS[g]
                sync.wait_ge(sem_ts, u + 1)
                if u >= 4:
                    sync.wait_ge(out_sems[u % 4], 16 * (u // 4))
                sync.dma_start(
                    out_d[m * P : (m + 1) * P, c0 : c0 + w],
                    stage[:, (u % 4) * GW : (u % 4) * GW + w],
                ).then_inc(out_sems[u % 4], 16)

        @block.tensor
        def _(tensor):
            tensor.wait_ge(in_l, 16)
            tensor.wait_ge(in_ext, 16)
            tensor.wait_ge(sem_z, 1)
            for g, (c0, w) in enumerate(GROUPS):
                for s in rhs_g_sems[g]:
                    tensor.wait_ge(s, 16)
                for m in range(MC):
                    u = g * MC + m
                    lsl = l_s[:, :, m * P : (m + 1) * P]
                    if u >= 2:
                        prev = u - 2
                        if prev % 4 == 3:
                            tensor.wait_ge(sem_dsq, prev // 4 + 1)
                        else:
                            tensor.wait_ge(sem_act, prev + 1 - (prev + 1) // 4)
                    inst = None
                    for j in range(w // NT):
                        n0 = c0 + j * NT
                        p0 = (u % 2) * GW + j * NT
                        tensor.matmul(
                            ps[:, p0 : p0 + NT],
                            lsl,
                            r_s[:, :, n0 : n0 + NT],
                            start=True,
                            stop=False,
                            perf_mode=PM,
                        )
                        inst = tensor.matmul(
                            ps[:, p0 : p0 + NT],
                            extw_s[:, :, :],
                            extr_s[:, :, n0 : n0 + NT],
                            start=False,
                            stop=True,
                            perf_mode=PM,
                        )
                    inst.then_inc(sem_mm, 1)

        @block.scalar
        def _(scalar):
            scalar.wait_ge(in_sc, 16)
            for u in range(NGI):
                if u % 4 == 3:
                    continue  # handled by the DVE sqrt bit-hack path
                g, m = divmod(u, MC)
                w = GROUPS[g][1]
                scalar.wait_ge(sem_mm, u + 1)
                if u >= 4:
                    scalar.wait_ge(sem_ts, u - 3)
                scalar.activation(
                    tbuf[:, (u % 4) * GW : (u % 4) * GW + w],
                    ps[:, (u % 2) * GW : (u % 2) * GW + w],
                    AF.Sqrt,
                    scale=sc_s[:, m : m + 1],
                    bias=sc_s[:, MC + m : MC + m + 1],
                ).then_inc(sem_act, 1)

        @block.vector
        def _(vector):
            import concourse.mybir as mybir
            vector.memset(
                extr_s[:, :, 0 : N // 2].bitcast(mybir.dt.uint32), 0
            ).then_inc(sem_z, 1)
            vector.memset(
                extr_s[:, :, N // 2 : N].bitcast(mybir.dt.uint32), 0
            ).then_inc(sem_z, 1)
            u16 = mybir.dt.uint16
            for u in range(NGI):
                g, m = divmod(u, MC)
                w = GROUPS[g][1]
                slot = u % 4
                tb = tbuf[:, slot * GW : slot * GW + w]
                st = stage[:, slot * GW : slot * GW + w]
                if slot == 3:
                    # DVE sqrt path: x = S^2*r2*(sq+eps) as fp16, then the
                    # (bits>>1)+C sqrt approximation (~2% rms, vs the 2e-2
                    # gate), then out = 1 - y/S.  Offloads 1/4 of the sqrt
                    # work from the ScalarE, which paces the kernel.
                    vector.wait_ge(sem_mm, u + 1)
                    vector.tensor_scalar(
                        tb,
                        ps[:, (u % 2) * GW : (u % 2) * GW + w],
                        sc_s[:, 2 * MC + m : 2 * MC + m + 1],
                        sc_s[:, 3 * MC + m : 3 * MC + m + 1],
                        op0=ALU.mult,
                        op1=ALU.add,
                    ).then_inc(sem_dsq, 1)
                    vector.tensor_scalar(
                        tb.bitcast(u16), tb.bitcast(u16), 1, None,
                        op0=ALU.logical_shift_right,
                    )
                    vector.tensor_scalar(
                        tb.bitcast(u16), tb.bitcast(u16), 0x1DDC, None,
                        op0=ALU.add,
                    )
                    if u >= 4:
                        vector.wait_ge(out_sems[slot], 16 * (u // 4))
                    vector.tensor_scalar(
                        st, tb, -1.0 / 1024.0, 1.0,
                        op0=ALU.mult, op1=ALU.add,
                    ).then_inc(sem_ts, 1)
                else:
                    vector.wait_ge(sem_act, u + 1 - (u + 1) // 4)
                    if u >= 4:
                        vector.wait_ge(out_sems[slot], 16 * (u // 4))
                    vector.tensor_scalar(
                        st, tb, -1.0, 1.0,
                        op0=ALU.mult, op1=ALU.add,
                    ).then_inc(sem_ts, 1)

    return nc


def _prep_inputs(p_u):
    a8 = p_u.astype(F8)
    af = a8.astype(np.float32)
    a64 = af.astype(np.float64)
    ni64 = np.einsum("ij,ij->i", a64, a64)

    njp = ni64 - CNJ
    hi8 = (njp / 16.0).astype(np.float32).astype(F8)
    hi = hi8.astype(np.float64)
    r = njp - 16.0 * hi
    mid8 = r.astype(np.float32).astype(F8)
    mid = mid8.astype(np.float64)
    lo8 = (16.0 * (r - mid)).astype(np.float32).astype(F8)
    lo = lo8.astype(np.float64)
    nj_eff = CNJ + 16.0 * hi + mid + lo / 16.0

    t64 = a64.sum(axis=0)
    rowsum = N * ni64 + nj_eff.sum() - 2.0 * (a64 @ t64) + N * EPS
    r2 = 1.0 / rowsum
    bias64 = r2 * (ni64 + CNJ + EPS)
    S2 = 1024.0 * 1024.0

    rt = np.ascontiguousarray(a8.T.reshape(2, P, N).transpose(1, 0, 2))
    extr = np.zeros((2, 2, N), dtype=F8)
    extr[0, 0] = hi8
    extr[0, 1] = mid8
    extr[1, 0] = lo8
    extw = np.zeros((P, 2, P), dtype=F8)
    extw[0, 0, :] = F8(16.0)
    extw[0, 1, :] = F8(1.0)
    extw[1, 0, :] = F8(1.0 / 16.0)

    m2 = (-2.0 * af).astype(F8)
    r2f = r2.astype(np.float32)
    biasf = bias64.astype(np.float32)

    in_maps = []
    for c in range(NCORES):
        sl = slice(c * M_PER_CORE, (c + 1) * M_PER_CORE)
        lt = np.ascontiguousarray(
            m2[sl].T.reshape(2, P, M_PER_CORE).transpose(1, 0, 2)
        )
        sc = np.concatenate(
            [
                np.ascontiguousarray(r2f[sl].reshape(MC, P).T),
                np.ascontiguousarray(biasf[sl].reshape(MC, P).T),
                np.ascontiguousarray(
                    (S2 * r2[sl]).astype(np.float32).reshape(MC, P).T
                ),
                np.ascontiguousarray(
                    (S2 * bias64[sl]).astype(np.float32).reshape(MC, P).T
                ),
            ],
            axis=1,
        ).astype(np.float32)
        in_maps.append({"lt": lt, "rt": rt, "extw": extw, "extr": extr, "sc": sc})
    return in_maps


def kernel(p_u):
    from concourse.bass_utils import run_bass_kernel_spmd

    p_u = np.asarray(p_u, dtype=np.float32)
    assert p_u.shape == (N, D)

    if "nc" not in _CACHE:
        _CACHE["nc"] = _build_nc()
    nc = _CACHE["nc"]

    in_maps = _prep_inputs(p_u)
    trace = bool(_CACHE.get("trace"))
    res = run_bass_kernel_spmd(nc, in_maps, core_ids=list(range(NCORES)), trace=trace)
    _CACHE["last_result"] = res
    out = np.concatenate(
        [res.results[c]["out"].astype(np.float32) for c in range(NCORES)], axis=0
    )
    return out


# revision 19
# speedup vs baseline: 1.0946x; 1.0946x over previous
"""PrefSimMat kernel, fp8 DoubleRow variant with UNIFORM pair-K=128 matmuls.

Both matmuls per tile are DoubleRow [128, 2, *] so the PE tile mode never
changes (the earlier fp8 attempts interleaved a K=2/3 matmul, which
reconfigured the PE row groups every tile and pinned the clock cold).
Features contract in ONE DoubleRow matmul (128 partitions x 2 pairs = 256
rows); the nj extension rides a second DoubleRow matmul with 3 live rows
(nj - 256 = 16*hi + mid + lo/16 in fp8e4) zero-padded to the same shape.
"""

import numpy as np
import ml_dtypes

F8 = ml_dtypes.float8_e4m3

N = 8192
D = 256
P = 128
NCORES = 8
M_PER_CORE = N // NCORES
MC = M_PER_CORE // P
NT = 512
GW = 2048
GROUPS = [(0, 2048), (2048, 2048), (4096, 2048), (6144, 2048)]
NG = len(GROUPS)
EPS = 2.0 ** -3
CNJ = 256.0

OUT_DT = np.float16

_CACHE = {}


def _build_nc():
    import concourse.bass as bass
    import concourse.mybir as mybir

    f32 = mybir.dt.float32
    f16 = mybir.dt.float16
    f8 = mybir.dt.float8e4
    AF = mybir.ActivationFunctionType
    ALU = mybir.AluOpType
    PM = mybir.MatmulPerfMode.DoubleRow

    nc = bass.Bass()
    l_d = nc.dram_tensor("lt", [P, 2, M_PER_CORE], f8, kind="ExternalInput")
    r_d = nc.dram_tensor("rt", [P, 2, N], f8, kind="ExternalInput")
    extw_d = nc.dram_tensor("extw", [P, 2, P], f8, kind="ExternalInput")
    extr_d = nc.dram_tensor("extr", [2, 2, N], f8, kind="ExternalInput")
    sc_d = nc.dram_tensor("sc", [P, 2 * MC], f32, kind="ExternalInput")
    out_d = nc.dram_tensor("out", [M_PER_CORE, N], f16, kind="ExternalOutput")

    NGI = MC * NG

    from contextlib import ExitStack

    with ExitStack() as ctx:
        r_s = ctx.enter_context(nc.sbuf_tensor("r_s", [P, 2, N], f8))
        l_s = ctx.enter_context(nc.sbuf_tensor("l_s", [P, 2, M_PER_CORE], f8))
        extw_s = ctx.enter_context(nc.sbuf_tensor("extw_s", [P, 2, P], f8))
        extr_s = ctx.enter_context(nc.sbuf_tensor("extr_s", [P, 2, N], f8))
        sc_s = ctx.enter_context(nc.sbuf_tensor("sc_s", [P, 2 * MC], f32))
        tbuf = ctx.enter_context(nc.sbuf_tensor("tbuf", [P, 4 * GW], f16))
        stage = ctx.enter_context(nc.sbuf_tensor("stage", [P, 4 * GW], f16))
        ps = ctx.enter_context(nc.psum_tensor("ps", [P, 2 * GW], f32))
        rhs_g_sems = [
            [ctx.enter_context(nc.semaphore(f"in_r{c}_{g}")) for c in range(2)]
            for g in range(NG)
        ]
        in_l = ctx.enter_context(nc.semaphore("in_l"))
        in_ext = ctx.enter_context(nc.semaphore("in_ext"))
        in_sc = ctx.enter_context(nc.semaphore("in_sc"))
        sem_mm = ctx.enter_context(nc.semaphore("sem_mm"))
        sem_act = ctx.enter_context(nc.semaphore("sem_act"))
        sem_ts = ctx.enter_context(nc.semaphore("sem_ts"))
        out_sems = [ctx.enter_context(nc.semaphore(f"dma_o{s}")) for s in range(4)]
        sem_z = ctx.enter_context(nc.semaphore("sem_z"))
        block = ctx.enter_context(nc.Block())

        @block.sync
        def _(sync):
            sync.dma_start(l_s[:, :, :], l_d[:, :, :]).then_inc(in_l, 16)
            sync.dma_start(extw_s[:, :, :], extw_d[:, :, :]).then_inc(in_ext, 16)
            for g, (c0, w) in enumerate(GROUPS):
                sync.dma_start(
                    r_s[:, :, c0 : c0 + w], r_d[:, :, c0 : c0 + w]
                ).then_inc(rhs_g_sems[g][0], 16)
                # only the 2 live ext rows come from DRAM; the whole tensor
                # is zeroed first by the DVE memset (quadrant-aligned)
                if g == 0:
                    sync.wait_ge(sem_z, 1)
                elif g == NG // 2:
                    sync.wait_ge(sem_z, 2)
                sync.dma_start(
                    extr_s[0:2, :, c0 : c0 + w], extr_d[:, :, c0 : c0 + w]
                ).then_inc(rhs_g_sems[g][1], 16)
                if g == 0:
                    sync.dma_start(sc_s[:, :], sc_d[:, :]).then_inc(in_sc, 16)
            for u in range(NGI):
                g, m = divmod(u, MC)
                c0, w = GROUPS[g]
                sync.wait_ge(sem_ts, u + 1)
                if u >= 4:
                    sync.wait_ge(out_sems[u % 4], 16 * (u // 4))
                sync.dma_start(
                    out_d[m * P : (m + 1) * P, c0 : c0 + w],
                    stage[:, (u % 4) * GW : (u % 4) * GW + w],
                ).then_inc(out_sems[u % 4], 16)

        @block.tensor
        def _(tensor):
            tensor.wait_ge(in_l, 16)
            tensor.wait_ge(in_ext, 16)
            tensor.wait_ge(sem_z, 1)
            for g, (c0, w) in enumerate(GROUPS):
                for s in rhs_g_sems[g]:
                    tensor.wait_ge(s, 16)
                for m in range(MC):
                    u = g * MC + m
                    lsl = l_s[:, :, m * P : (m + 1) * P]
                    if u >= 2:
                        tensor.wait_ge(sem_act, u - 1)
                    inst = None
                    for j in range(w // NT):
                        n0 = c0 + j * NT
                        p0 = (u % 2) * GW + j * NT
                        tensor.matmul(
                            ps[:, p0 : p0 + NT],
                            lsl,
                            r_s[:, :, n0 : n0 + NT],
                            start=True,
                            stop=False,
                            perf_mode=PM,
                        )
                        inst = tensor.matmul(
                            ps[:, p0 : p0 + NT],
                            extw_s[:, :, :],
                            extr_s[:, :, n0 : n0 + NT],
                            start=False,
                            stop=True,
                            perf_mode=PM,
                        )
                    inst.then_inc(sem_mm, 1)

        @block.scalar
        def _(scalar):
            scalar.wait_ge(in_sc, 16)
            for u in range(NGI):
                g, m = divmod(u, MC)
                w = GROUPS[g][1]
                scalar.wait_ge(sem_mm, u + 1)
                if u >= 4:
                    scalar.wait_ge(sem_ts, u - 3)
                scalar.activation(
                    tbuf[:, (u % 4) * GW : (u % 4) * GW + w],
                    ps[:, (u % 2) * GW : (u % 2) * GW + w],
                    AF.Sqrt,
                    scale=sc_s[:, m : m + 1],
                    bias=sc_s[:, MC + m : MC + m + 1],
                ).then_inc(sem_act, 1)

        @block.vector
        def _(vector):
            import concourse.mybir as mybir
            vector.memset(
                extr_s[:, :, 0 : N // 2].bitcast(mybir.dt.uint32), 0
            ).then_inc(sem_z, 1)
            vector.memset(
                extr_s[:, :, N // 2 : N].bitcast(mybir.dt.uint32), 0
            ).then_inc(sem_z, 1)
            for u in range(NGI):
                g = u // MC
                w = GROUPS[g][1]
                vector.wait_ge(sem_act, u + 1)
                if u >= 4:
                    vector.wait_ge(out_sems[u % 4], 16 * (u // 4))
                vector.tensor_scalar(
                    stage[:, (u % 4) * GW : (u % 4) * GW + w],
                    tbuf[:, (u % 4) * GW : (u % 4) * GW + w],
                    -1.0,
                    1.0,
                    op0=ALU.mult,
                    op1=ALU.add,
                ).then_inc(sem_ts, 1)

    return nc


def _prep_inputs(p_u):
    a8 = p_u.astype(F8)
    af = a8.astype(np.float32)
    a64 = af.astype(np.float64)
    ni64 = np.einsum("ij,ij->i", a64, a64)

    njp = ni64 - CNJ
    hi8 = (njp / 16.0).astype(np.float32).astype(F8)
    hi = hi8.astype(np.float64)
    r = njp - 16.0 * hi
    mid8 = r.astype(np.float32).astype(F8)
    mid = mid8.astype(np.float64)
    lo8 = (16.0 * (r - mid)).astype(np.float32).astype(F8)
    lo = lo8.astype(np.float64)
    nj_eff = CNJ + 16.0 * hi + mid + lo / 16.0

    t64 = a64.sum(axis=0)
    rowsum = N * ni64 + nj_eff.sum() - 2.0 * (a64 @ t64) + N * EPS
    r2 = 1.0 / rowsum
    bias64 = r2 * (ni64 + CNJ + EPS)

    rt = np.ascontiguousarray(a8.T.reshape(2, P, N).transpose(1, 0, 2))
    extr = np.zeros((2, 2, N), dtype=F8)
    extr[0, 0] = hi8
    extr[0, 1] = mid8
    extr[1, 0] = lo8
    extw = np.zeros((P, 2, P), dtype=F8)
    extw[0, 0, :] = F8(16.0)
    extw[0, 1, :] = F8(1.0)
    extw[1, 0, :] = F8(1.0 / 16.0)

    m2 = (-2.0 * af).astype(F8)
    r2f = r2.astype(np.float32)
    biasf = bias64.astype(np.float32)

    in_maps = []
    for c in range(NCORES):
        sl = slice(c * M_PER_CORE, (c + 1) * M_PER_CORE)
        lt = np.ascontiguousarray(
            m2[sl].T.reshape(2, P, M_PER_CORE).transpose(1, 0, 2)
        )
        sc = np.concatenate(
            [
                np.ascontiguousarray(r2f[sl].reshape(MC, P).T),
                np.ascontiguousarray(biasf[sl].reshape(MC, P).T),
            ],
            axis=1,
        ).astype(np.float32)
        in_maps.append({"lt": lt, "rt": rt, "extw": extw, "extr": extr, "sc": sc})
    return in_maps


def kernel(p_u):
    from concourse.bass_utils import run_bass_kernel_spmd

    p_u = np.asarray(p_u, dtype=np.float32)
    assert p_u.shape == (N, D)

    if "nc" not in _CACHE:
        _CACHE["nc"] = _build_nc()
    nc = _CACHE["nc"]

    in_maps = _prep_inputs(p_u)
    trace = bool(_CACHE.get("trace"))
    res = run_bass_kernel_spmd(nc, in_maps, core_ids=list(range(NCORES)), trace=trace)
    _CACHE["last_result"] = res
    out = np.concatenate(
        [res.results[c]["out"].astype(np.float32) for c in range(NCORES)], axis=0
    )
    return out
